# revision 1
# baseline (speedup 1.0000x reference)
"""ALiBi transformer layer on 8 TRN2 NeuronCores.

Sharding: token-parallel. 4096 tokens split 512/core; cores 0-3 own batch 0,
cores 4-7 own batch 1. Weights replicated (pre-transposed + LN-gain-folded to
bf16 on host). Only collective: a 4-rank AllGather of (k^T, v') per batch
group, issued right after the k/v projections and overlapped with the q
projection.

On-chip layout is transposed throughout: activations are [feature->partitions,
tokens->free], so every matmul consumes natural operands (contraction dim on
partitions) and no on-chip transposes are needed:
  - LN stats  = ones-vector matmuls over partition tiles (+ DVE combines)
  - rsqrt     = exp(-0.5*ln(var+eps)) -- keeps ACT on one table set (ln+exp)
  - scores^T  = kT_tile.T @ qT  -> psum[j,i]; head pairs packed in the PE
                array via row groups (K=64 each)
  - ALiBi     = per-key bias slope*(j-(S-1)) (softmax shift-invariance kills
                the -slope*i term) applied for free as the exp ACT bias
                (j is the partition dim of scores^T)
  - softmax   = un-normalized exp; denominator comes from a ones column
                appended to v (65-row AV matmul output), normalization folded
                into a per-head reciprocal broadcast after AV
  - AV        = v'_tile.T @ E^T accumulated over 16 j-tiles
"""

import math
import sys

import numpy as np

try:  # make concourse importable regardless of harness sys.path
    import concourse  # noqa: F401
except ImportError:
    for _p in ("/opt/trn_rl_repo", "/root/.axon_site/_ro/trn_rl_repo"):
        if _p not in sys.path:
            sys.path.insert(0, _p)

B, S, D, H, F = 2, 2048, 1024, 16, 4096
HD = D // H  # 64
EPS = 1e-5
NCORES = 8
GROUP = 4  # cores per batch group
T = (B * S) // NCORES  # 512 tokens per core
NT = T  # alias
DT = D // 128  # 8 partition tiles of the model dim
FT = F // 128  # 32 partition tiles of the ff dim
JT = S // 128  # 16 key tiles per batch
VPC = H * (HD + 1)  # 1040 v' columns (64 v + 1 ones per head)
KT_ELEMS = D * T  # kT block elems in the AG payload
VP_ELEMS = T * VPC  # v' block elems
CC_ELEMS = KT_ELEMS + VP_ELEMS

_CACHE = {}
DEBUG = False  # adds intermediate-dump outputs


def _alibi_slopes(n_heads):
    start = 2.0 ** (-(2.0 ** (-(math.log2(n_heads) - 3))))
    return np.array([start * (start**i) for i in range(n_heads)], dtype=np.float32)


def _build_nc():
    import concourse.bass as bass
    import concourse.mybir as mybir
    import concourse.tile as tile
    from concourse import bacc
    from contextlib import ExitStack

    f32 = mybir.dt.float32
    bf16 = mybir.dt.bfloat16
    Alu = mybir.AluOpType
    Act = mybir.ActivationFunctionType

    nc = bacc.Bacc("TRN2", num_devices=NCORES)

    # ---- I/O -----------------------------------------------------------
    xT = nc.declare_dram_parameter("xT", [D, T], f32, isOutput=False)
    wqT = nc.declare_dram_parameter("wqT", [D, D], bf16, isOutput=False)
    wkT = nc.declare_dram_parameter("wkT", [D, D], bf16, isOutput=False)
    wvT = nc.declare_dram_parameter("wvT", [D, D], bf16, isOutput=False)
    woT = nc.declare_dram_parameter("woT", [D, D], bf16, isOutput=False)
    w1T = nc.declare_dram_parameter("w1T", [D, F], bf16, isOutput=False)
    w2T = nc.declare_dram_parameter("w2T", [F, D], bf16, isOutput=False)
    bq = nc.declare_dram_parameter("bq", [128, DT], f32, isOutput=False)
    bk = nc.declare_dram_parameter("bk", [128, DT], f32, isOutput=False)
    bvr = nc.declare_dram_parameter("bvr", [1, D], bf16, isOutput=False)
    bo = nc.declare_dram_parameter("bo", [128, DT], f32, isOutput=False)
    b1 = nc.declare_dram_parameter("b1", [128, FT], f32, isOutput=False)
    b2 = nc.declare_dram_parameter("b2", [128, DT], f32, isOutput=False)
    alibi = nc.declare_dram_parameter("alibi", [128, JT * H], f32, isOutput=False)
    out = nc.declare_dram_parameter("out", [D, T], f32, isOutput=True)
    dbg = {}
    if DEBUG:
        dbg["xhat"] = nc.declare_dram_parameter("d_xhat", [D, T], bf16, isOutput=True)
        dbg["qt"] = nc.declare_dram_parameter("d_qt", [D, T], bf16, isOutput=True)
        dbg["ccout"] = nc.declare_dram_parameter(
            "d_ccout", [GROUP, CC_ELEMS], bf16, isOutput=True
        )
        dbg["e0"] = nc.declare_dram_parameter("d_e0", [128, T], bf16, isOutput=True)
        dbg["den"] = nc.declare_dram_parameter("d_den", [1, T], f32, isOutput=True)
        dbg["ao"] = nc.declare_dram_parameter("d_ao", [64, T], bf16, isOutput=True)
        dbg["x2"] = nc.declare_dram_parameter("d_x2", [128, T], f32, isOutput=True)
        dbg["aoall"] = nc.declare_dram_parameter(
            "d_aoall", [H, 64, T], bf16, isOutput=True
        )
        dbg["denall"] = nc.declare_dram_parameter(
            "d_denall", [H, T], f32, isOutput=True
        )
        dbg["rball"] = nc.declare_dram_parameter(
            "d_rball", [H, T], f32, isOutput=True
        )
        dbg["wo0"] = nc.declare_dram_parameter("d_wo0", [128, T], f32, isOutput=True)
        dbg["sc1"] = nc.declare_dram_parameter(
            "d_sc1", [4, 128, T], f32, isOutput=True
        )
        dbg["e1"] = nc.declare_dram_parameter(
            "d_e1", [4, 128, T], bf16, isOutput=True
        )

    groups = [[0, 1, 2, 3], [4, 5, 6, 7]]

    with ExitStack() as top:
        tc = top.enter_context(tile.TileContext(nc))

        # ---- persistent pools -----------------------------------------
        const = top.enter_context(tc.tile_pool(name="const", bufs=1))
        xt_pool = top.enter_context(tc.tile_pool(name="xt_pool", bufs=DT))
        xhat_pool = top.enter_context(tc.tile_pool(name="xhat_pool", bufs=DT))
        qt_pool = top.enter_context(tc.tile_pool(name="qt_pool", bufs=DT))
        ao_pool = top.enter_context(tc.tile_pool(name="ao_pool", bufs=H // 2))
        x2_pool = top.enter_context(tc.tile_pool(name="x2_pool", bufs=DT))
        ff1_pool = top.enter_context(tc.tile_pool(name="ff1_pool", bufs=FT))
        wt_pool = top.enter_context(tc.tile_pool(name="wt_pool", bufs=10))
        dram = top.enter_context(tc.tile_pool(name="dram", bufs=1, space="DRAM"))

        # ---- constants -------------------------------------------------
        ones_col = const.tile([128, 1], f32)  # stats matmul lhsT
        nc.vector.memset(ones_col, 1.0)
        ones_row = const.tile([1, 128], f32)  # row->[128,*] broadcast lhsT
        nc.vector.memset(ones_row, 1.0)
        ones_hrow = const.tile([HD + 1, 64], f32)  # denom broadcast lhsT (row 64)
        nc.vector.memset(ones_hrow, 1.0)
        ones_vb = const.tile([128, H], bf16)  # ones cols of v' bounce
        nc.vector.memset(ones_vb, 1.0)
        ones_rowb = const.tile([1, 128], bf16)  # v-bias matmul lhsT
        nc.vector.memset(ones_rowb, 1.0)
        eps_ap = const.tile([1, 1], f32)
        nc.vector.memset(eps_ap, EPS)
        zero64 = const.tile([HD + 1, 1], f32)  # zero bias AP at base 64
        nc.vector.memset(zero64, 0.0)

        alibi_sb = const.tile([128, JT * H], f32)
        nc.sync.dma_start(out=alibi_sb, in_=alibi[:, :])
        bq_sb = const.tile([128, DT], f32)
        nc.sync.dma_start(out=bq_sb, in_=bq[:, :])
        bk_sb = const.tile([128, DT], f32)
        nc.sync.dma_start(out=bk_sb, in_=bk[:, :])
        bvr_sb = const.tile([1, D], bf16)
        nc.sync.dma_start(out=bvr_sb, in_=bvr[:, :])
        bo_sb = const.tile([128, DT], f32)
        nc.sync.dma_start(out=bo_sb, in_=bo[:, :])
        b1_sb = const.tile([128, FT], f32)
        nc.sync.dma_start(out=b1_sb, in_=b1[:, :])
        b2_sb = const.tile([128, DT], f32)
        nc.sync.dma_start(out=b2_sb, in_=b2[:, :])

        # ---- AG bounce buffers (k and v gathered separately so scores
        # can start as soon as k lands) ---------------------------------
        cc_in_k = dram.tile([KT_ELEMS], bf16)
        cc_in_v = dram.tile([VP_ELEMS], bf16)
        cc_out_k = dram.tile([GROUP, KT_ELEMS], bf16)
        cc_out_v = dram.tile([GROUP, VP_ELEMS], bf16)
        ccin_k = cc_in_k[:].rearrange("(d t) -> d t", t=T)
        ccin_v = cc_in_v[:].rearrange("(t c) -> t c", c=VPC)

        # ---------------------------------------------------------------
        def layernorm_T(xt_tiles, psum_pool, tmp_pool, out_dtype):
            """Transposed layernorm: per-token (free axis) mean/var via
            ones-matmuls over the 8 partition tiles; returns 8 normalized
            tiles in out_dtype."""
            ps_sum = psum_pool.tile([1, T], f32, name="ps_sum", tag="ln")
            ps_sq = psum_pool.tile([1, T], f32, name="ps_sq", tag="ln")
            for t in range(DT):
                sq = tmp_pool.tile([128, T], f32, name="sq", tag="sq")
                nc.vector.tensor_mul(sq, xt_tiles[t], xt_tiles[t])
                nc.tensor.matmul(
                    ps_sum, lhsT=ones_col, rhs=xt_tiles[t],
                    start=(t == 0), stop=(t == DT - 1),
                )
                nc.tensor.matmul(
                    ps_sq, lhsT=ones_col, rhs=sq,
                    start=(t == 0), stop=(t == DT - 1),
                )
            mu = tmp_pool.tile([1, T], f32, name="mu", tag="lnrow")
            nc.scalar.mul(out=mu, in_=ps_sum, mul=1.0 / D)
            e2 = tmp_pool.tile([1, T], f32, name="e2", tag="lnrow")
            nc.scalar.mul(out=e2, in_=ps_sq, mul=1.0 / D)
            var = tmp_pool.tile([1, T], f32, name="var", tag="lnrow")
            nc.vector.tensor_mul(var, mu, mu)
            nc.vector.tensor_sub(var, e2, var)
            lnv = tmp_pool.tile([1, T], f32, name="lnv", tag="lnrow")
            nc.scalar.activation(lnv, var, Act.Ln, bias=eps_ap, scale=1.0)
            rr = tmp_pool.tile([1, T], f32, name="rr", tag="lnrow")
            nc.scalar.activation(rr, lnv, Act.Exp, scale=-0.5)
            br = tmp_pool.tile([1, T], f32, name="br", tag="lnrow")
            nc.vector.tensor_mul(br, mu, rr)
            ps_a = psum_pool.tile([128, T], f32, name="ps_a", tag="ln")
            ps_b = psum_pool.tile([128, T], f32, name="ps_b", tag="ln")
            nc.tensor.matmul(ps_a, lhsT=ones_row, rhs=rr, start=True, stop=True)
            nc.tensor.matmul(ps_b, lhsT=ones_row, rhs=br, start=True, stop=True)
            a_sb = tmp_pool.tile([128, T], f32, name="a_sb", tag="lnab")
            b_sb = tmp_pool.tile([128, T], f32, name="b_sb", tag="lnab")
            nc.vector.tensor_copy(a_sb, ps_a)
            nc.vector.tensor_copy(b_sb, ps_b)
            outs = []
            for t in range(DT):
                tmp = tmp_pool.tile([128, T], f32, name="lntmp", tag="sq")
                nc.vector.tensor_mul(tmp, xt_tiles[t], a_sb)
                xh = xhat_pool.tile([128, T], out_dtype, name="xh", tag="xh")
                nc.vector.tensor_sub(xh, tmp, b_sb)
                outs.append(xh)
            return outs

        # ---- phase 1: load x, LN1 -------------------------------------
        xt_tiles = []
        for t in range(DT):
            xt = xt_pool.tile([128, T], f32, name="xt", tag="xt")
            nc.sync.dma_start(out=xt, in_=xT[t * 128:(t + 1) * 128, :])
            xt_tiles.append(xt)

        with tc.tile_pool(name="ln1_ps", bufs=2, space="PSUM") as ln_ps, \
             tc.tile_pool(name="ln1_tmp", bufs=3) as ln_tmp, \
             tc.tile_pool(name="proj_ps", bufs=6, space="PSUM") as proj_ps, \
             tc.tile_pool(name="ev_tmp", bufs=3) as ev_tmp:
            xhat = layernorm_T(xt_tiles, ln_ps, ln_tmp, bf16)

            # ---- k projection -> bounce (4-wide out groups, slab DMA) -
            for og in range(2):
                pss = [
                    proj_ps.tile([128, T], f32, name=f"ps_k{oi}", tag="proj", bufs=6)
                    for oi in range(4)
                ]
                for kt in range(DT):
                    w = wt_pool.tile([128, 512], bf16, name="wk_t", tag="wsl")
                    nc.scalar.dma_start(
                        out=w,
                        in_=wkT[kt * 128:(kt + 1) * 128, og * 512:(og + 1) * 512],
                    )
                    for oi in range(4):
                        nc.tensor.matmul(
                            pss[oi], lhsT=w[:, oi * 128:(oi + 1) * 128],
                            rhs=xhat[kt], start=(kt == 0), stop=(kt == DT - 1),
                        )
                for oi in range(4):
                    ot = og * 4 + oi
                    kt_sb = ev_tmp.tile([128, T], bf16, name="kt_sb", tag="ev")
                    nc.vector.tensor_scalar(
                        out=kt_sb, in0=pss[oi], scalar1=bk_sb[:, ot:ot + 1],
                        scalar2=None, op0=Alu.add,
                    )
                    nc.sync.dma_start(
                        out=ccin_k[ot * 128:(ot + 1) * 128, :], in_=kt_sb
                    )

            # ---- AllGather of k (overlaps v/q projections) ------------
            nc.gpsimd.collective_compute(
                "AllGather",
                Alu.bypass,
                replica_groups=groups,
                ins=[cc_in_k[:]],
                outs=[cc_out_k[:]],
            )

            # ---- v projection (normal layout) -> bounce ---------------
            for oh in range(2):
                psv = [
                    proj_ps.tile([128, 512], f32, name=f"ps_v{it}", tag="proj", bufs=6)
                    for it in range(4)
                ]
                for kt in range(DT):
                    w = wt_pool.tile([128, 512], bf16, name="wv_t", tag="wsl")
                    nc.scalar.dma_start(
                        out=w,
                        in_=wvT[kt * 128:(kt + 1) * 128, oh * 512:(oh + 1) * 512],
                    )
                    for it in range(4):
                        nc.tensor.matmul(
                            psv[it], lhsT=xhat[kt][:, it * 128:(it + 1) * 128],
                            rhs=w, start=(kt == 0), stop=False,
                        )
                for it in range(4):
                    nc.tensor.matmul(
                        psv[it], lhsT=ones_rowb,
                        rhs=bvr_sb[:, oh * 512:(oh + 1) * 512],
                        start=False, stop=True,
                    )
                    v_sb = ev_tmp.tile([128, 512], bf16, name="v_sb", tag="ev")
                    nc.vector.tensor_copy(v_sb, psv[it])
                    dst = ccin_v[it * 128:(it + 1) * 128, :].rearrange(
                        "t (h c) -> t h c", c=HD + 1
                    )[:, oh * 8:(oh + 1) * 8, 0:HD]
                    src = v_sb.rearrange("t (h c) -> t h c", c=HD)
                    nc.sync.dma_start(out=dst, in_=src)
                    if oh == 0:
                        # ones columns of v' (written once per row block)
                        dst1 = ccin_v[it * 128:(it + 1) * 128, :].rearrange(
                            "t (h c) -> t h c", c=HD + 1
                        )[:, :, HD:HD + 1]
                        nc.sync.dma_start(
                            out=dst1, in_=ones_vb.rearrange("t (h o) -> t h o", o=1)
                        )

            # ---- AllGather of v (overlaps q projection + scores) ------
            nc.gpsimd.collective_compute(
                "AllGather",
                Alu.bypass,
                replica_groups=groups,
                ins=[cc_in_v[:]],
                outs=[cc_out_v[:]],
            )

            if DEBUG:
                nc.sync.dma_start(
                    out=dbg["ccout"][:, 0:KT_ELEMS], in_=cc_out_k[:, :]
                )
                nc.sync.dma_start(
                    out=dbg["ccout"][:, KT_ELEMS:CC_ELEMS], in_=cc_out_v[:, :]
                )

            # ---- q projection (local only) ----------------------------
            qt_tiles = []
            for og in range(2):
                psq = [
                    proj_ps.tile([128, T], f32, name=f"ps_q{oi}", tag="proj",
                                 bufs=6)
                    for oi in range(4)
                ]
                for kt in range(DT):
                    w = wt_pool.tile([128, 512], bf16, name="wq_t", tag="wsl")
                    nc.scalar.dma_start(
                        out=w,
                        in_=wqT[kt * 128:(kt + 1) * 128, og * 512:(og + 1) * 512],
                    )
                    for oi in range(4):
                        nc.tensor.matmul(
                            psq[oi], lhsT=w[:, oi * 128:(oi + 1) * 128],
                            rhs=xhat[kt], start=(kt == 0), stop=(kt == DT - 1),
                        )
                for oi in range(4):
                    ot = og * 4 + oi
                    qt = qt_pool.tile([128, T], bf16, name="qt", tag="qt")
                    nc.vector.tensor_scalar(
                        out=qt, in0=psq[oi], scalar1=bq_sb[:, ot:ot + 1],
                        scalar2=None, op0=Alu.add,
                    )
                    qt_tiles.append(qt)
            if DEBUG:
                for t in range(DT):
                    nc.sync.dma_start(
                        out=dbg["xhat"][t * 128:(t + 1) * 128, :], in_=xhat[t]
                    )
                    nc.sync.dma_start(
                        out=dbg["qt"][t * 128:(t + 1) * 128, :], in_=qt_tiles[t]
                    )

        # ---- phase 2: attention ---------------------------------------
        ao_tiles = []  # 16 tiles [64, T] bf16, head-major
        with tc.tile_pool(name="sc_ps", bufs=4, space="PSUM") as sc_ps, \
             tc.tile_pool(name="av_ps", bufs=2, space="PSUM") as av_ps, \
             tc.tile_pool(name="rb_ps", bufs=2, space="PSUM") as rb_ps, \
             tc.tile_pool(name="at_sb", bufs=4) as at_sb, \
             tc.tile_pool(name="dn_sb", bufs=1) as dn_sb, \
             tc.tile_pool(name="rb_sb", bufs=2) as rb_sbp:
            for hp in range(H // 2):
                ps_e = av_ps.tile([128, T], f32, name="ps_e", tag="av", bufs=3)
                ps_o = av_ps.tile([128, T], f32, name="ps_o", tag="av", bufs=3)
                # whole-rank k slabs for this head pair (one DMA per rank)
                kt_slabs = []
                for r in range(GROUP):
                    ck = cc_out_k[r, :].rearrange("(d t) -> d t", t=T)
                    ks = at_sb.tile([128, T], bf16, name="ks", tag="ktt", bufs=8)
                    nc.sync.dma_start(
                        out=ks, in_=ck[hp * 128:(hp + 1) * 128, :]
                    )
                    kt_slabs.append(ks)
                for jt in range(JT):
                    r, jl = jt // 4, jt % 4
                    cv = cc_out_v[r, :].rearrange("(t c) -> t c", c=VPC)
                    kt_t = kt_slabs[r][:, jl * 128:(jl + 1) * 128]
                    vp_t = at_sb.tile([128, 2 * (HD + 1)], bf16, name="vp_t", tag="vpt")
                    nc.sync.dma_start(
                        out=vp_t,
                        in_=cv[jl * 128:(jl + 1) * 128,
                               hp * 2 * (HD + 1):(hp + 1) * 2 * (HD + 1)],
                    )
                    ps_se = sc_ps.tile([128, T], f32, name="ps_se", tag="sc", bufs=3)
                    nc.tensor.matmul(
                        ps_se, lhsT=kt_t[0:64, :], rhs=qt_tiles[hp][0:64, :],
                        start=True, stop=True,
                    )
                    ps_so = sc_ps.tile([128, T], f32, name="ps_so", tag="sc", bufs=3)
                    nc.tensor.matmul(
                        ps_so, lhsT=kt_t[64:128, :], rhs=qt_tiles[hp][64:128, :],
                        start=True, stop=True,
                    )
                    e_e = at_sb.tile([128, T], bf16, name="e_e", tag="ee")
                    nc.scalar.activation(
                        e_e, ps_se, Act.Exp,
                        bias=alibi_sb[:, jt * H + 2 * hp:jt * H + 2 * hp + 1],
                        scale=1.0,
                    )
                    e_o = at_sb.tile([128, T], bf16, name="e_o", tag="eo")
                    nc.scalar.activation(
                        e_o, ps_so, Act.Exp,
                        bias=alibi_sb[:, jt * H + 2 * hp + 1:jt * H + 2 * hp + 2],
                        scale=1.0,
                    )
                    if DEBUG and hp == 0 and jt == 0:
                        nc.sync.dma_start(out=dbg["e0"][:, :], in_=e_e)
                    if DEBUG and hp == 1 and jt < 4:
                        sc_cp = at_sb.tile([128, T], f32, name="sc_cp", tag="scc")
                        nc.vector.tensor_copy(sc_cp, ps_se)
                        nc.sync.dma_start(out=dbg["sc1"][jt], in_=sc_cp)
                        nc.sync.dma_start(out=dbg["e1"][jt], in_=e_e)
                    nc.tensor.matmul(
                        ps_e[0:HD + 1, :], lhsT=vp_t[:, 0:HD + 1], rhs=e_e,
                        start=(jt == 0), stop=(jt == JT - 1),
                    )
                    nc.tensor.matmul(
                        ps_o[0:HD + 1, :], lhsT=vp_t[:, HD + 1:2 * (HD + 1)], rhs=e_o,
                        start=(jt == 0), stop=(jt == JT - 1),
                    )
                # normalize the two heads of this pair; pack them into one
                # [128, T] tile (odd head partition-shifted via SBUF DMA) so
                # the Wo projection runs full-K matmuls
                ao_pair = ao_pool.tile([128, T], bf16, name="ao_pair", tag="ao")
                for which, ps in ((0, ps_e), (1, ps_o)):
                    h = 2 * hp + which
                    den_all = dn_sb.tile(
                        [HD + 1, T], f32, name="den_all", tag="den", bufs=2
                    )
                    rden_all = dn_sb.tile(
                        [HD + 1, T], f32, name="rden_all", tag="rden", bufs=2
                    )
                    nc.vector.tensor_copy(den_all[HD:HD + 1, :], ps[HD:HD + 1, :])
                    if DEBUG and h == 0:
                        nc.sync.dma_start(
                            out=dbg["den"][:, :], in_=den_all[HD:HD + 1, :]
                        )
                    lnd = dn_sb.tile(
                        [HD + 1, T], f32, name="lnd", tag="lnd", bufs=2
                    )
                    nc.scalar.activation(
                        lnd[HD:HD + 1, :], den_all[HD:HD + 1, :], Act.Ln,
                        bias=zero64[HD:HD + 1, :], scale=1.0,
                    )
                    nc.scalar.activation(
                        rden_all[HD:HD + 1, :], lnd[HD:HD + 1, :], Act.Exp,
                        scale=-1.0,
                    )
                    ps_rb = rb_ps.tile([64, T], f32, name="ps_rb", tag="rb")
                    nc.tensor.matmul(
                        ps_rb, lhsT=ones_hrow[HD:HD + 1, :],
                        rhs=rden_all[HD:HD + 1, :],
                        start=True, stop=True,
                    )
                    rb = rb_sbp.tile([64, T], f32, name="rb", tag="rbs")
                    nc.vector.tensor_copy(rb, ps_rb)
                    if which == 0:
                        nc.vector.tensor_mul(ao_pair[0:HD, :], ps[0:HD, :], rb)
                    else:
                        ao_tmp = rb_sbp.tile(
                            [64, T], bf16, name="ao_tmp", tag="aot", bufs=2
                        )
                        nc.vector.tensor_mul(ao_tmp, ps[0:HD, :], rb)
                        nc.sync.dma_start(out=ao_pair[HD:128, :], in_=ao_tmp)
                    if DEBUG:
                        src = ao_pair[0:HD, :] if which == 0 else ao_pair[HD:128, :]
                        if h == 0:
                            nc.sync.dma_start(out=dbg["ao"][:, :], in_=src)
                        nc.sync.dma_start(out=dbg["aoall"][h, :, :], in_=src)
                        nc.sync.dma_start(
                            out=dbg["denall"][h:h + 1, :], in_=den_all[HD:HD + 1, :]
                        )
                ao_tiles.append(ao_pair)

        # ---- phase 3: output projection + residual --------------------
        x2_tiles = [None] * DT
        with tc.tile_pool(name="wo_ps", bufs=4, space="PSUM") as wo_ps:
            for og in range(2):
                psw = [
                    wo_ps.tile([128, T], f32, name=f"ps_wo{oi}", tag="wo", bufs=8)
                    for oi in range(4)
                ]
                for hp in range(H // 2):
                    w = wt_pool.tile([128, 512], bf16, name="wo_t", tag="wsl")
                    nc.scalar.dma_start(
                        out=w,
                        in_=woT[hp * 128:(hp + 1) * 128, og * 512:(og + 1) * 512],
                    )
                    for oi in range(4):
                        nc.tensor.matmul(
                            psw[oi], lhsT=w[:, oi * 128:(oi + 1) * 128],
                            rhs=ao_tiles[hp], start=(hp == 0),
                            stop=(hp == H // 2 - 1),
                        )
                for oi in range(4):
                    ot = og * 4 + oi
                    x2 = x2_pool.tile([128, T], f32, name="x2", tag="x2")
                    nc.vector.tensor_scalar(
                        out=x2, in0=psw[oi], scalar1=bo_sb[:, ot:ot + 1],
                        scalar2=None, op0=Alu.add,
                    )
                    if DEBUG and ot == 0:
                        nc.sync.dma_start(out=dbg["wo0"][:, :], in_=x2)
                    nc.vector.tensor_add(x2, x2, xt_tiles[ot])
                    x2_tiles[ot] = x2
                    if DEBUG and ot == 0:
                        nc.sync.dma_start(out=dbg["x2"][:, :], in_=x2)

        # ---- phase 4: LN2 + FFN ---------------------------------------
        with tc.tile_pool(name="ln2_ps", bufs=2, space="PSUM") as ln2_ps, \
             tc.tile_pool(name="ln2_tmp", bufs=3) as ln2_tmp:
            xhat2 = layernorm_T(x2_tiles, ln2_ps, ln2_tmp, bf16)

            ff1_tiles = []
            with tc.tile_pool(name="f1_ps", bufs=3, space="PSUM") as f1_ps:
                for fg in range(FT // 4):
                    psf = [
                        f1_ps.tile([128, T], f32, name=f"ps_f1{fi}", tag="f1",
                                   bufs=6)
                        for fi in range(4)
                    ]
                    for kt in range(DT):
                        w = wt_pool.tile([128, 512], bf16, name="w1_t", tag="wsl")
                        nc.scalar.dma_start(
                            out=w,
                            in_=w1T[kt * 128:(kt + 1) * 128,
                                    fg * 512:(fg + 1) * 512],
                        )
                        for fi in range(4):
                            nc.tensor.matmul(
                                psf[fi], lhsT=w[:, fi * 128:(fi + 1) * 128],
                                rhs=xhat2[kt],
                                start=(kt == 0), stop=(kt == DT - 1),
                            )
                    for fi in range(4):
                        ft = fg * 4 + fi
                        f1 = ff1_pool.tile([128, T], bf16, name="f1", tag="f1s")
                        nc.scalar.activation(
                            f1, psf[fi], Act.Relu, bias=b1_sb[:, ft:ft + 1],
                            scale=1.0,
                        )
                        ff1_tiles.append(f1)

            # second FFN matmul: two groups of 4 output tiles, accumulate
            # over all 32 f-tiles with w2 streamed once per group
            with tc.tile_pool(name="f2_ps", bufs=4, space="PSUM") as f2_ps:
                for og in range(2):
                    pss = []
                    for oi in range(4):
                        ps = f2_ps.tile([128, T], f32, name="ps_f2", tag="f2")
                        pss.append(ps)
                    for ft in range(FT):
                        w = wt_pool.tile([128, 512], bf16, name="w2_t", tag="wsl")
                        nc.scalar.dma_start(
                            out=w,
                            in_=w2T[ft * 128:(ft + 1) * 128,
                                    og * 512:(og + 1) * 512],
                        )
                        for oi in range(4):
                            nc.tensor.matmul(
                                pss[oi], lhsT=w[:, oi * 128:(oi + 1) * 128],
                                rhs=ff1_tiles[ft],
                                start=(ft == 0), stop=(ft == FT - 1),
                            )
                    for oi in range(4):
                        ot = og * 4 + oi
                        y = ln2_tmp.tile([128, T], f32, name="y", tag="sq")
                        nc.vector.tensor_scalar(
                            out=y, in0=pss[oi], scalar1=b2_sb[:, ot:ot + 1],
                            scalar2=None, op0=Alu.add,
                        )
                        nc.vector.tensor_add(y, y, x2_tiles[ot])
                        nc.sync.dma_start(
                            out=out[ot * 128:(ot + 1) * 128, :], in_=y
                        )

    nc.compile()
    return nc


def _get_nc():
    if "nc" not in _CACHE:
        _CACHE["nc"] = _build_nc()
    return _CACHE["nc"]


def kernel(x, Wq, Wk, Wv, Wo, bo, W1, b1, W2, b2, g1, be1, g2, be2):
    import ml_dtypes

    f32 = np.float32
    bf = ml_dtypes.bfloat16
    x = np.asarray(x, f32)
    Wq = np.asarray(Wq, f32); Wk = np.asarray(Wk, f32)
    Wv = np.asarray(Wv, f32); Wo = np.asarray(Wo, f32)
    W1 = np.asarray(W1, f32); W2 = np.asarray(W2, f32)
    bo = np.asarray(bo, f32); b1 = np.asarray(b1, f32); b2 = np.asarray(b2, f32)
    g1 = np.asarray(g1, f32); be1 = np.asarray(be1, f32)
    g2 = np.asarray(g2, f32); be2 = np.asarray(be2, f32)

    scale = 1.0 / math.sqrt(HD)
    wqT = np.ascontiguousarray((Wq * g1[None, :] * scale).T).astype(bf)
    wkT = np.ascontiguousarray((Wk * g1[None, :]).T).astype(bf)
    wvT = np.ascontiguousarray((Wv * g1[None, :]).T).astype(bf)
    woT = np.ascontiguousarray(Wo.T).astype(bf)
    w1T = np.ascontiguousarray((W1 * g2[None, :]).T).astype(bf)
    w2T = np.ascontiguousarray(W2.T).astype(bf)
    bq_v = (be1 @ Wq.T) * scale
    bk_v = be1 @ Wk.T
    bv_v = be1 @ Wv.T
    b1_v = b1 + be2 @ W1.T

    def cols(v, nt):  # (nt*128,) -> (128, nt) [partition, tile]
        return np.ascontiguousarray(v.reshape(nt, 128).T).astype(f32)

    slopes = _alibi_slopes(H)
    j = np.arange(S, dtype=f32)
    vals = slopes[:, None] * (j[None, :] - (S - 1))  # (H, S)
    alibi = np.ascontiguousarray(
        vals.reshape(H, JT, 128).transpose(2, 1, 0).reshape(128, JT * H)
    ).astype(f32)

    xt_flat = x.reshape(B * S, D)
    base = {
        "wqT": wqT, "wkT": wkT, "wvT": wvT, "woT": woT,
        "w1T": w1T, "w2T": w2T,
        "bq": cols(bq_v, DT), "bk": cols(bk_v, DT),
        "bvr": np.ascontiguousarray(bv_v[None, :]).astype(bf),
        "bo": cols(bo, DT), "b1": cols(b1_v, FT), "b2": cols(b2, DT),
        "alibi": alibi,
    }
    in_maps = []
    for c in range(NCORES):
        m = dict(base)
        m["xT"] = np.ascontiguousarray(xt_flat[c * T:(c + 1) * T].T).astype(f32)
        in_maps.append(m)

    from concourse.bass_utils import run_bass_kernel_spmd

    nc = _get_nc()
    res = run_bass_kernel_spmd(nc, in_maps, core_ids=list(range(NCORES)))
    _CACHE["last_result"] = res
    outs = [r["out"] for r in res.results]  # each (D, T)
    full = np.empty((B * S, D), dtype=f32)
    for c in range(NCORES):
        full[c * T:(c + 1) * T] = outs[c].T
    return full.reshape(B, S, D)



# revision 4
# speedup vs baseline: 1.5302x; 1.5302x over previous
"""ALiBi transformer layer on 8 TRN2 NeuronCores.

Sharding: token-parallel. 4096 tokens split 512/core; cores 0-3 own batch 0,
cores 4-7 own batch 1. Weights replicated (pre-transposed + LN-gain-folded to
bf16 on host). Collectives: three 4-rank AllGathers per batch group —
kT heads 0-7, kT heads 8-15, then v' — so scores start as soon as the first
k half lands and the exp work hides the v wire time.

On-chip layout is transposed throughout: activations are [feature->partitions,
tokens->free] so every matmul consumes natural operands:
  - LN stats  = ones-vector matmuls over partition tiles (+ DVE combines)
  - rsqrt     = exp(-0.5*ln(var+eps)) -- keeps ACT on one table set (ln+exp)
  - k/q/v are quantized to fp8-e4m3 at PSUM eviction with power-of-2 scales
    folded into the host-side weights (k,v: x32; q: x8). The AllGather
    payload is fp8 (half the wire time of bf16).
  - scores^T  = kT_tile.T @ qT -> psum[j,i], fp8 x fp8, scaled 2^11
  - ALiBi     = per-key bias slope*(j-(S-1)) applied as the exp ACT bias;
                exp scale 2^-11 undoes the fp8 scaling. exp output is fp8.
  - softmax   = un-normalized; denominator from a ones column appended to v'
                (65-row AV matmul), normalized via DVE reciprocal + a K=1
                f16 broadcast matmul after AV.
  - Banding: ALiBi slopes make keys far from the sequence end numerically
    irrelevant; head pair p only visits its last PT[p] key tiles
    (PT = [1,1,1,2,4,7,14,16], margin ~14 nats, verified ~4.6e-3 rel err).
  - Attention is two phases: all scores+exp (stashed fp8 e tiles), then all
    AV+normalize — PE never stalls on the v AllGather.
"""

import math
import sys

import numpy as np

try:  # make concourse importable regardless of harness sys.path
    import concourse  # noqa: F401
except ImportError:
    for _p in ("/opt/trn_rl_repo", "/root/.axon_site/_ro/trn_rl_repo"):
        if _p not in sys.path:
            sys.path.insert(0, _p)

B, S, D, H, F = 2, 2048, 1024, 16, 4096
HD = D // H  # 64
EPS = 1e-5
NCORES = 8
GROUP = 4  # cores per batch group
T = (B * S) // NCORES  # 512 tokens per core
DT = D // 128  # 8 partition tiles of the model dim
FT = F // 128  # 32 partition tiles of the ff dim
JT = S // 128  # 16 key tiles per batch
VPC = H * (HD + 1)  # 1040 v' columns (64 v + 1 ones per head)
KH_ELEMS = (D // 2) * T  # kT half-block elems in one AG payload
VP_ELEMS = T * VPC  # v' block elems

# band tiles per head PAIR (max of the two heads), margin ~14 nats
PT = [1, 1, 1, 2, 4, 7, 14, 16]

QS = 8.0  # extra q scale (on top of 1/sqrt(HD))
KS = 32.0  # k scale
VS = 32.0  # v scale
ESCALE = 1.0 / (QS * KS)  # undone inside the exp activation

_CACHE = {}


def _alibi_slopes(n_heads):
    start = 2.0 ** (-(2.0 ** (-(math.log2(n_heads) - 3))))
    return np.array([start * (start**i) for i in range(n_heads)], dtype=np.float32)


def _build_nc():
    import concourse.bass as bass  # noqa: F401
    import concourse.mybir as mybir
    import concourse.tile as tile
    from concourse import bacc
    from contextlib import ExitStack

    f32 = mybir.dt.float32
    f16 = mybir.dt.float16
    bf16 = mybir.dt.bfloat16
    f8 = mybir.dt.float8e4
    Alu = mybir.AluOpType
    Act = mybir.ActivationFunctionType

    nc = bacc.Bacc("TRN2", num_devices=NCORES)

    # ---- I/O -----------------------------------------------------------
    xT = nc.declare_dram_parameter("xT", [D, T], f32, isOutput=False)
    wqT = nc.declare_dram_parameter("wqT", [D, D], bf16, isOutput=False)
    wkT = nc.declare_dram_parameter("wkT", [D, D], bf16, isOutput=False)
    wvT = nc.declare_dram_parameter("wvT", [D, D], bf16, isOutput=False)
    woT = nc.declare_dram_parameter("woT", [D, D], bf16, isOutput=False)
    w1T = nc.declare_dram_parameter("w1T", [D, F], bf16, isOutput=False)
    w2T = nc.declare_dram_parameter("w2T", [F, D], bf16, isOutput=False)
    bq = nc.declare_dram_parameter("bq", [128, DT], f32, isOutput=False)
    bk = nc.declare_dram_parameter("bk", [128, DT], f32, isOutput=False)
    bvr = nc.declare_dram_parameter("bvr", [1, D], bf16, isOutput=False)
    bo = nc.declare_dram_parameter("bo", [128, DT], f32, isOutput=False)
    b1 = nc.declare_dram_parameter("b1", [128, FT], f32, isOutput=False)
    b2 = nc.declare_dram_parameter("b2", [128, DT], f32, isOutput=False)
    alibi = nc.declare_dram_parameter("alibi", [128, JT * H], f32, isOutput=False)
    out = nc.declare_dram_parameter("out", [D, T], f32, isOutput=True)

    groups = [[0, 1, 2, 3], [4, 5, 6, 7]]

    with ExitStack() as top:
        tc = top.enter_context(tile.TileContext(nc))

        # ---- persistent pools -----------------------------------------
        const = top.enter_context(tc.tile_pool(name="const", bufs=1))
        xt_pool = top.enter_context(tc.tile_pool(name="xt_pool", bufs=DT))
        xhat_pool = top.enter_context(tc.tile_pool(name="xhat_pool", bufs=DT))
        qt_pool = top.enter_context(tc.tile_pool(name="qt_pool", bufs=DT))
        e_pool = top.enter_context(tc.tile_pool(name="e_pool", bufs=2 * sum(PT)))
        vt_pool = top.enter_context(tc.tile_pool(name="vt_pool", bufs=16))
        kt_pool = top.enter_context(tc.tile_pool(name="kt_pool", bufs=6))
        ao_pool = top.enter_context(tc.tile_pool(name="ao_pool", bufs=H // 2))
        x2_pool = top.enter_context(tc.tile_pool(name="x2_pool", bufs=DT))
        ff1_pool = top.enter_context(tc.tile_pool(name="ff1_pool", bufs=FT))
        wt_pool = top.enter_context(tc.tile_pool(name="wt_pool", bufs=12))
        wt2_pool = top.enter_context(tc.tile_pool(name="wt2_pool", bufs=12))
        dram = top.enter_context(tc.tile_pool(name="dram", bufs=1, space="DRAM"))

        # ---- constants -------------------------------------------------
        ones_col = const.tile([128, 1], f32)  # stats matmul lhsT
        nc.vector.memset(ones_col, 1.0)
        ones_row = const.tile([1, 128], f32)  # row->[128,*] broadcast lhsT
        nc.vector.memset(ones_row, 1.0)
        ones_hrow = const.tile([HD + 1, 64], f16)  # denom broadcast lhsT (row 64)
        nc.vector.memset(ones_hrow, 1.0)
        ones_rowb = const.tile([1, 128], bf16)  # v-bias matmul lhsT
        nc.vector.memset(ones_rowb, 1.0)
        eps_ap = const.tile([1, 1], f32)
        nc.vector.memset(eps_ap, EPS)

        alibi_sb = const.tile([128, JT * H], f32)
        nc.sync.dma_start(out=alibi_sb, in_=alibi[:, :])
        bq_sb = const.tile([128, DT], f32)
        nc.sync.dma_start(out=bq_sb, in_=bq[:, :])
        bk_sb = const.tile([128, DT], f32)
        nc.sync.dma_start(out=bk_sb, in_=bk[:, :])
        bvr_sb = const.tile([1, D], bf16)
        nc.sync.dma_start(out=bvr_sb, in_=bvr[:, :])
        bo_sb = const.tile([128, DT], f32)
        nc.sync.dma_start(out=bo_sb, in_=bo[:, :])
        b1_sb = const.tile([128, FT], f32)
        nc.sync.dma_start(out=b1_sb, in_=b1[:, :])
        b2_sb = const.tile([128, DT], f32)
        nc.sync.dma_start(out=b2_sb, in_=b2[:, :])

        # ---- AG bounce buffers ----------------------------------------
        cc_in_k0 = dram.tile([KH_ELEMS], f8)
        cc_in_k1 = dram.tile([KH_ELEMS], f8)
        cc_in_v = dram.tile([VP_ELEMS], f8)
        cc_out_k0 = dram.tile([GROUP, KH_ELEMS], f8)
        cc_out_k1 = dram.tile([GROUP, KH_ELEMS], f8)
        cc_out_v = dram.tile([GROUP, VP_ELEMS], f8)
        ccin_k = [
            cc_in_k0[:].rearrange("(d t) -> d t", t=T),
            cc_in_k1[:].rearrange("(d t) -> d t", t=T),
        ]
        ccin_v = cc_in_v[:].rearrange("(t c) -> t c", c=VPC)

        # ---------------------------------------------------------------
        def layernorm_T(xt_tiles, psum_pool, tmp_pool, out_pool, out_dtype):
            """Transposed layernorm: per-token (free axis) mean/var via
            ones-matmuls over the 8 partition tiles; returns 8 normalized
            tiles in out_dtype."""
            ps_sum = psum_pool.tile([1, T], f32, name="ps_sum", tag="ln")
            ps_sq = psum_pool.tile([1, T], f32, name="ps_sq", tag="ln")
            for t in range(DT):
                sq = tmp_pool.tile([128, T], f32, name="sq", tag="sq")
                nc.vector.tensor_mul(sq, xt_tiles[t], xt_tiles[t])
                nc.tensor.matmul(
                    ps_sum, lhsT=ones_col, rhs=xt_tiles[t],
                    start=(t == 0), stop=(t == DT - 1),
                )
                nc.tensor.matmul(
                    ps_sq, lhsT=ones_col, rhs=sq,
                    start=(t == 0), stop=(t == DT - 1),
                )
            mu = tmp_pool.tile([1, T], f32, name="mu", tag="lnrow")
            nc.scalar.mul(out=mu, in_=ps_sum, mul=1.0 / D)
            e2 = tmp_pool.tile([1, T], f32, name="e2", tag="lnrow")
            nc.scalar.mul(out=e2, in_=ps_sq, mul=1.0 / D)
            var = tmp_pool.tile([1, T], f32, name="var", tag="lnrow")
            nc.vector.tensor_mul(var, mu, mu)
            nc.vector.tensor_sub(var, e2, var)
            lnv = tmp_pool.tile([1, T], f32, name="lnv", tag="lnrow")
            nc.scalar.activation(lnv, var, Act.Ln, bias=eps_ap, scale=1.0)
            rr = tmp_pool.tile([1, T], f32, name="rr", tag="lnrow")
            nc.scalar.activation(rr, lnv, Act.Exp, scale=-0.5)
            br = tmp_pool.tile([1, T], f32, name="br", tag="lnrow")
            nc.vector.tensor_mul(br, mu, rr)
            ps_a = psum_pool.tile([128, T], f32, name="ps_a", tag="ln")
            ps_b = psum_pool.tile([128, T], f32, name="ps_b", tag="ln")
            nc.tensor.matmul(ps_a, lhsT=ones_row, rhs=rr, start=True, stop=True)
            nc.tensor.matmul(ps_b, lhsT=ones_row, rhs=br, start=True, stop=True)
            a_sb = tmp_pool.tile([128, T], f32, name="a_sb", tag="lnab")
            b_sb = tmp_pool.tile([128, T], f32, name="b_sb", tag="lnab")
            nc.vector.tensor_copy(a_sb, ps_a)
            nc.vector.tensor_copy(b_sb, ps_b)
            outs = []
            for t in range(DT):
                tmp = tmp_pool.tile([128, T], f32, name="lntmp", tag="sq")
                nc.vector.tensor_mul(tmp, xt_tiles[t], a_sb)
                xh = out_pool.tile([128, T], out_dtype, name="xh", tag="xh")
                nc.vector.tensor_sub(xh, tmp, b_sb)
                outs.append(xh)
            return outs

        # ---- phase 1: load x, LN1 -------------------------------------
        xt_tiles = []
        for t in range(DT):
            xt = xt_pool.tile([128, T], f32, name="xt", tag="xt")
            nc.sync.dma_start(out=xt, in_=xT[t * 128:(t + 1) * 128, :])
            xt_tiles.append(xt)

        with tc.tile_pool(name="ln1_ps", bufs=2, space="PSUM") as ln_ps, \
             tc.tile_pool(name="ln1_tmp", bufs=3) as ln_tmp, \
             tc.tile_pool(name="proj_ps", bufs=6, space="PSUM") as proj_ps, \
             tc.tile_pool(name="ev_tmp", bufs=3) as ev_tmp:
            xhat = layernorm_T(xt_tiles, ln_ps, ln_tmp, xhat_pool, bf16)

            # ---- k projection -> bounce, one AllGather per og half ----
            for og in range(2):
                pss = [
                    proj_ps.tile([128, T], f32, name=f"ps_k{oi}", tag="proj", bufs=6)
                    for oi in range(4)
                ]
                for kt in range(DT):
                    w = wt_pool.tile([128, 512], bf16, name="wk_t", tag="wsl")
                    nc.scalar.dma_start(
                        out=w,
                        in_=wkT[kt * 128:(kt + 1) * 128, og * 512:(og + 1) * 512],
                    )
                    for oi in range(4):
                        nc.tensor.matmul(
                            pss[oi], lhsT=w[:, oi * 128:(oi + 1) * 128],
                            rhs=xhat[kt], start=(kt == 0), stop=(kt == DT - 1),
                        )
                for oi in range(4):
                    ot = og * 4 + oi
                    kt_sb = ev_tmp.tile([128, T], f8, name="kt_sb", tag="ev")
                    nc.vector.tensor_scalar(
                        out=kt_sb, in0=pss[oi], scalar1=bk_sb[:, ot:ot + 1],
                        scalar2=None, op0=Alu.add,
                    )
                    nc.sync.dma_start(
                        out=ccin_k[og][oi * 128:(oi + 1) * 128, :], in_=kt_sb
                    )
                nc.gpsimd.collective_compute(
                    "AllGather",
                    Alu.bypass,
                    replica_groups=groups,
                    ins=[(cc_in_k0 if og == 0 else cc_in_k1)[:]],
                    outs=[(cc_out_k0 if og == 0 else cc_out_k1)[:]],
                )

            # ---- v projection (token-partition layout) -> bounce ------
            for oh in range(2):
                psv = [
                    proj_ps.tile([128, 512], f32, name=f"ps_v{it}", tag="proj", bufs=6)
                    for it in range(4)
                ]
                for kt in range(DT):
                    w = wt_pool.tile([128, 512], bf16, name="wv_t", tag="wsl")
                    nc.scalar.dma_start(
                        out=w,
                        in_=wvT[kt * 128:(kt + 1) * 128, oh * 512:(oh + 1) * 512],
                    )
                    for it in range(4):
                        nc.tensor.matmul(
                            psv[it], lhsT=xhat[kt][:, it * 128:(it + 1) * 128],
                            rhs=w, start=(kt == 0), stop=False,
                        )
                for it in range(4):
                    nc.tensor.matmul(
                        psv[it], lhsT=ones_rowb,
                        rhs=bvr_sb[:, oh * 512:(oh + 1) * 512],
                        start=False, stop=True,
                    )
                    # stage with per-head ones columns interleaved: [t, 8, 65]
                    vst = ev_tmp.tile([128, 8 * (HD + 1)], f8, name="vst", tag="vst")
                    vst3 = vst.rearrange("t (h c) -> t h c", c=HD + 1)
                    nc.vector.tensor_copy(
                        vst3[:, :, 0:HD],
                        psv[it].rearrange("t (h c) -> t h c", c=HD),
                    )
                    nc.vector.memset(vst3[:, :, HD:HD + 1], 1.0)
                    nc.sync.dma_start(
                        out=ccin_v[it * 128:(it + 1) * 128,
                                   oh * 8 * (HD + 1):(oh + 1) * 8 * (HD + 1)],
                        in_=vst,
                    )
            nc.gpsimd.collective_compute(
                "AllGather",
                Alu.bypass,
                replica_groups=groups,
                ins=[cc_in_v[:]],
                outs=[cc_out_v[:]],
            )

            # ---- q projection (local only) ----------------------------
            qt_tiles = []
            for og in range(2):
                psq = [
                    proj_ps.tile([128, T], f32, name=f"ps_q{oi}", tag="proj",
                                 bufs=6)
                    for oi in range(4)
                ]
                for kt in range(DT):
                    w = wt_pool.tile([128, 512], bf16, name="wq_t", tag="wsl")
                    nc.scalar.dma_start(
                        out=w,
                        in_=wqT[kt * 128:(kt + 1) * 128, og * 512:(og + 1) * 512],
                    )
                    for oi in range(4):
                        nc.tensor.matmul(
                            psq[oi], lhsT=w[:, oi * 128:(oi + 1) * 128],
                            rhs=xhat[kt], start=(kt == 0), stop=(kt == DT - 1),
                        )
                for oi in range(4):
                    ot = og * 4 + oi
                    qt = qt_pool.tile([128, T], f8, name="qt", tag="qt")
                    nc.vector.tensor_scalar(
                        out=qt, in0=psq[oi], scalar1=bq_sb[:, ot:ot + 1],
                        scalar2=None, op0=Alu.add,
                    )
                    qt_tiles.append(qt)

        # ---- phase 2a: scores + exp for every pair (e stashed fp8) ----
        e_tiles = {}  # (hp, jt, parity) -> fp8 [128, T]
        with tc.tile_pool(name="sc_ps", bufs=4, space="PSUM") as sc_ps:
            for hp in range(H // 2):
                nt = PT[hp]
                jt0 = JT - nt
                cko = cc_out_k0 if hp < 4 else cc_out_k1
                row0 = (hp % 4) * 128
                ks = {}
                for r in range(jt0 // 4, GROUP):
                    ck = cko[r, :].rearrange("(d t) -> d t", t=T)
                    ksl = kt_pool.tile([128, T], f8, name="ks", tag="ktt")
                    nc.sync.dma_start(out=ksl, in_=ck[row0:row0 + 128, :])
                    ks[r] = ksl
                for jt in range(jt0, JT):
                    r, jl = jt // 4, jt % 4
                    kt_t = ks[r][:, jl * 128:(jl + 1) * 128]
                    ps_se = sc_ps.tile([128, T], f32, name="ps_se", tag="sc")
                    nc.tensor.matmul(
                        ps_se, lhsT=kt_t[0:64, :], rhs=qt_tiles[hp][0:64, :],
                        start=True, stop=True,
                    )
                    ps_so = sc_ps.tile([128, T], f32, name="ps_so", tag="sc")
                    nc.tensor.matmul(
                        ps_so, lhsT=kt_t[64:128, :], rhs=qt_tiles[hp][64:128, :],
                        start=True, stop=True,
                    )
                    e_e = e_pool.tile([128, T], f8, name="e_e", tag="e")
                    nc.scalar.activation(
                        e_e, ps_se, Act.Exp,
                        bias=alibi_sb[:, jt * H + 2 * hp:jt * H + 2 * hp + 1],
                        scale=ESCALE,
                    )
                    e_o = e_pool.tile([128, T], f8, name="e_o", tag="e")
                    nc.scalar.activation(
                        e_o, ps_so, Act.Exp,
                        bias=alibi_sb[:, jt * H + 2 * hp + 1:jt * H + 2 * hp + 2],
                        scale=ESCALE,
                    )
                    e_tiles[(hp, jt, 0)] = e_e
                    e_tiles[(hp, jt, 1)] = e_o

        # ---- phase 2b: bulk-load gathered v' --------------------------
        v_tiles = []  # global key tile jt -> [128, VPC] fp8
        for r in range(GROUP):
            cv = cc_out_v[r, :].rearrange("(t c) -> t c", c=VPC)
            for jl in range(4):
                vt = vt_pool.tile([128, VPC], f8, name="vt", tag="vt")
                nc.sync.dma_start(out=vt, in_=cv[jl * 128:(jl + 1) * 128, :])
                v_tiles.append(vt)

        # ---- phase 2c: AV + normalize ---------------------------------
        ao_tiles = []  # 8 tiles [128, T] bf16, head-pair-major
        with tc.tile_pool(name="av_ps", bufs=4, space="PSUM") as av_ps, \
             tc.tile_pool(name="rb_ps", bufs=2, space="PSUM") as rb_ps, \
             tc.tile_pool(name="dn_sb", bufs=2) as dn_sb, \
             tc.tile_pool(name="rb_sb", bufs=2) as rb_sbp:
            for hp in range(H // 2):
                nt = PT[hp]
                jt0 = JT - nt
                ps_e = av_ps.tile([HD + 1, T], f32, name="ps_e", tag="av")
                ps_o = av_ps.tile([HD + 1, T], f32, name="ps_o", tag="av")
                for jt in range(jt0, JT):
                    vt = v_tiles[jt]
                    nc.tensor.matmul(
                        ps_e,
                        lhsT=vt[:, (2 * hp) * (HD + 1):(2 * hp + 1) * (HD + 1)],
                        rhs=e_tiles[(hp, jt, 0)],
                        start=(jt == jt0), stop=(jt == JT - 1),
                    )
                    nc.tensor.matmul(
                        ps_o,
                        lhsT=vt[:, (2 * hp + 1) * (HD + 1):(2 * hp + 2) * (HD + 1)],
                        rhs=e_tiles[(hp, jt, 1)],
                        start=(jt == jt0), stop=(jt == JT - 1),
                    )
                ao_pair = ao_pool.tile([128, T], bf16, name="ao_pair", tag="ao")
                for which, ps in ((0, ps_e), (1, ps_o)):
                    rden = dn_sb.tile([HD + 1, T], f32, name="rden", tag="rden")
                    nc.vector.reciprocal(rden[HD:HD + 1, :], ps[HD:HD + 1, :])
                    rdenh = dn_sb.tile([HD + 1, T], f16, name="rdenh", tag="rdenh")
                    nc.vector.tensor_copy(rdenh[HD:HD + 1, :], rden[HD:HD + 1, :])
                    ps_rb = rb_ps.tile([64, T], f32, name="ps_rb", tag="rb")
                    nc.tensor.matmul(
                        ps_rb, lhsT=ones_hrow[HD:HD + 1, :],
                        rhs=rdenh[HD:HD + 1, :],
                        start=True, stop=True,
                    )
                    rb = rb_sbp.tile([64, T], f32, name="rb", tag="rbs")
                    nc.vector.tensor_copy(rb, ps_rb)
                    if which == 0:
                        nc.vector.tensor_mul(ao_pair[0:HD, :], ps[0:HD, :], rb)
                    else:
                        ao_tmp = rb_sbp.tile(
                            [64, T], bf16, name="ao_tmp", tag="aot"
                        )
                        nc.vector.tensor_mul(ao_tmp, ps[0:HD, :], rb)
                        nc.sync.dma_start(out=ao_pair[HD:128, :], in_=ao_tmp)
                ao_tiles.append(ao_pair)

        # ---- phase 3: output projection + residual --------------------
        x2_tiles = [None] * DT
        with tc.tile_pool(name="wo_ps", bufs=4, space="PSUM") as wo_ps:
            for og in range(2):
                psw = [
                    wo_ps.tile([128, T], f32, name=f"ps_wo{oi}", tag="wo", bufs=8)
                    for oi in range(4)
                ]
                for hp in range(H // 2):
                    w = wt2_pool.tile([128, 512], bf16, name="wo_t", tag="wsl")
                    nc.gpsimd.dma_start(
                        out=w,
                        in_=woT[hp * 128:(hp + 1) * 128, og * 512:(og + 1) * 512],
                    )
                    for oi in range(4):
                        nc.tensor.matmul(
                            psw[oi], lhsT=w[:, oi * 128:(oi + 1) * 128],
                            rhs=ao_tiles[hp], start=(hp == 0),
                            stop=(hp == H // 2 - 1),
                        )
                for oi in range(4):
                    ot = og * 4 + oi
                    x2 = x2_pool.tile([128, T], f32, name="x2", tag="x2")
                    nc.vector.tensor_scalar(
                        out=x2, in0=psw[oi], scalar1=bo_sb[:, ot:ot + 1],
                        scalar2=None, op0=Alu.add,
                    )
                    nc.vector.tensor_add(x2, x2, xt_tiles[ot])
                    x2_tiles[ot] = x2

        # ---- phase 4: LN2 + FFN ---------------------------------------
        with tc.tile_pool(name="ln2_ps", bufs=2, space="PSUM") as ln2_ps, \
             tc.tile_pool(name="ln2_tmp", bufs=3) as ln2_tmp, \
             tc.tile_pool(name="xh2_pool", bufs=DT) as xh2_pool:
            xhat2 = layernorm_T(x2_tiles, ln2_ps, ln2_tmp, xh2_pool, bf16)

            ff1_tiles = []
            with tc.tile_pool(name="f1_ps", bufs=3, space="PSUM") as f1_ps:
                for fg in range(FT // 4):
                    psf = [
                        f1_ps.tile([128, T], f32, name=f"ps_f1{fi}", tag="f1",
                                   bufs=6)
                        for fi in range(4)
                    ]
                    for kt in range(DT):
                        w = wt2_pool.tile([128, 512], bf16, name="w1_t", tag="wsl")
                        nc.gpsimd.dma_start(
                            out=w,
                            in_=w1T[kt * 128:(kt + 1) * 128,
                                    fg * 512:(fg + 1) * 512],
                        )
                        for fi in range(4):
                            nc.tensor.matmul(
                                psf[fi], lhsT=w[:, fi * 128:(fi + 1) * 128],
                                rhs=xhat2[kt],
                                start=(kt == 0), stop=(kt == DT - 1),
                            )
                    for fi in range(4):
                        ft = fg * 4 + fi
                        f1 = ff1_pool.tile([128, T], bf16, name="f1", tag="f1s")
                        # relu(x + b1) fused on DVE (keeps ACT free)
                        nc.vector.tensor_scalar(
                            out=f1, in0=psf[fi], scalar1=b1_sb[:, ft:ft + 1],
                            scalar2=0.0, op0=Alu.add, op1=Alu.max,
                        )
                        ff1_tiles.append(f1)

            # second FFN matmul: two groups of 4 output tiles, accumulate
            # over all 32 f-tiles with w2 streamed once per group
            with tc.tile_pool(name="f2_ps", bufs=4, space="PSUM") as f2_ps:
                for og in range(2):
                    pss = []
                    for oi in range(4):
                        ps = f2_ps.tile([128, T], f32, name="ps_f2", tag="f2")
                        pss.append(ps)
                    for ft in range(FT):
                        w = wt2_pool.tile([128, 512], bf16, name="w2_t", tag="wsl")
                        nc.gpsimd.dma_start(
                            out=w,
                            in_=w2T[ft * 128:(ft + 1) * 128,
                                    og * 512:(og + 1) * 512],
                        )
                        for oi in range(4):
                            nc.tensor.matmul(
                                pss[oi], lhsT=w[:, oi * 128:(oi + 1) * 128],
                                rhs=ff1_tiles[ft],
                                start=(ft == 0), stop=(ft == FT - 1),
                            )
                    for oi in range(4):
                        ot = og * 4 + oi
                        y = ln2_tmp.tile([128, T], f32, name="y", tag="sq")
                        nc.vector.tensor_scalar(
                            out=y, in0=pss[oi], scalar1=b2_sb[:, ot:ot + 1],
                            scalar2=None, op0=Alu.add,
                        )
                        nc.vector.tensor_add(y, y, x2_tiles[ot])
                        nc.sync.dma_start(
                            out=out[ot * 128:(ot + 1) * 128, :], in_=y
                        )

    nc.compile()
    return nc


def _get_nc():
    if "nc" not in _CACHE:
        _CACHE["nc"] = _build_nc()
    return _CACHE["nc"]


def kernel(x, Wq, Wk, Wv, Wo, bo, W1, b1, W2, b2, g1, be1, g2, be2):
    import ml_dtypes

    f32 = np.float32
    bf = ml_dtypes.bfloat16
    x = np.asarray(x, f32)
    Wq = np.asarray(Wq, f32); Wk = np.asarray(Wk, f32)
    Wv = np.asarray(Wv, f32); Wo = np.asarray(Wo, f32)
    W1 = np.asarray(W1, f32); W2 = np.asarray(W2, f32)
    bo = np.asarray(bo, f32); b1 = np.asarray(b1, f32); b2 = np.asarray(b2, f32)
    g1 = np.asarray(g1, f32); be1 = np.asarray(be1, f32)
    g2 = np.asarray(g2, f32); be2 = np.asarray(be2, f32)

    scale = 1.0 / math.sqrt(HD)
    wqT = np.ascontiguousarray((Wq * g1[None, :] * (scale * QS)).T).astype(bf)
    wkT = np.ascontiguousarray((Wk * g1[None, :] * KS).T).astype(bf)
    wvT = np.ascontiguousarray((Wv * g1[None, :] * VS).T).astype(bf)
    woT = np.ascontiguousarray(Wo.T / VS).astype(bf)
    w1T = np.ascontiguousarray((W1 * g2[None, :]).T).astype(bf)
    w2T = np.ascontiguousarray(W2.T).astype(bf)
    bq_v = (be1 @ Wq.T) * scale * QS
    bk_v = (be1 @ Wk.T) * KS
    bv_v = (be1 @ Wv.T) * VS
    b1_v = b1 + be2 @ W1.T

    def cols(v, nt):  # (nt*128,) -> (128, nt) [partition, tile]
        return np.ascontiguousarray(v.reshape(nt, 128).T).astype(f32)

    slopes = _alibi_slopes(H)
    j = np.arange(S, dtype=f32)
    vals = slopes[:, None] * (j[None, :] - (S - 1))  # (H, S)
    alibi = np.ascontiguousarray(
        vals.reshape(H, JT, 128).transpose(2, 1, 0).reshape(128, JT * H)
    ).astype(f32)

    xt_flat = x.reshape(B * S, D)
    base = {
        "wqT": wqT, "wkT": wkT, "wvT": wvT, "woT": woT,
        "w1T": w1T, "w2T": w2T,
        "bq": cols(bq_v, DT), "bk": cols(bk_v, DT),
        "bvr": np.ascontiguousarray(bv_v[None, :]).astype(bf),
        "bo": cols(bo, DT), "b1": cols(b1_v, FT), "b2": cols(b2, DT),
        "alibi": alibi,
    }
    in_maps = []
    for c in range(NCORES):
        m = dict(base)
        m["xT"] = np.ascontiguousarray(xt_flat[c * T:(c + 1) * T].T).astype(f32)
        in_maps.append(m)

    from concourse.bass_utils import run_bass_kernel_spmd

    nc = _get_nc()
    res = run_bass_kernel_spmd(nc, in_maps, core_ids=list(range(NCORES)))
    _CACHE["last_result"] = res
    outs = [r["out"] for r in res.results]  # each (D, T)
    full = np.empty((B * S, D), dtype=f32)
    for c in range(NCORES):
        full[c * T:(c + 1) * T] = outs[c].T
    return full.reshape(B, S, D)


# revision 11
# speedup vs baseline: 1.7804x; 1.1635x over previous
"""ALiBi transformer layer on 8 TRN2 NeuronCores.

Sharding: token-parallel. 4096 tokens split 512/core; cores 0-3 own batch 0,
cores 4-7 own batch 1. Weights replicated (pre-transposed + LN-gain-folded to
bf16 on host). Collectives: three 4-rank AllGathers per batch group —
kT heads 0-7, kT heads 8-15, then v' — so scores start as soon as the first
k half lands and the exp work hides the v wire time.

On-chip layout is transposed throughout: activations are [feature->partitions,
tokens->free] so every matmul consumes natural operands:
  - LN stats  = ones-vector matmuls over partition tiles (+ DVE combines)
  - rsqrt     = exp(-0.5*ln(var+eps)) -- keeps ACT on one table set (ln+exp)
  - k/q/v are quantized to fp8-e4m3 at PSUM eviction with power-of-2 scales
    folded into the host-side weights (k,v: x32; q: x8). The AllGather
    payload is fp8 (half the wire time of bf16).
  - scores^T  = kT_tile.T @ qT -> psum[j,i], fp8 x fp8, scaled 2^11
  - ALiBi     = per-key bias slope*(j-(S-1)) applied as the exp ACT bias;
                exp scale 2^-11 undoes the fp8 scaling. exp output is fp8.
  - softmax   = un-normalized; denominator from a ones column appended to v'
                (65-row AV matmul), normalized via DVE reciprocal + a K=1
                f16 broadcast matmul after AV.
  - Banding: ALiBi slopes make keys far from the sequence end numerically
    irrelevant; head pair p only visits its last PT[p] key tiles
    (PT = [1,1,1,2,4,7,14,16], margin ~14 nats, verified ~4.6e-3 rel err).
  - Attention is two phases: all scores+exp (stashed fp8 e tiles), then all
    AV+normalize — PE never stalls on the v AllGather.
"""

import math
import sys

import numpy as np

try:  # make concourse importable regardless of harness sys.path
    import concourse  # noqa: F401
except ImportError:
    for _p in ("/opt/trn_rl_repo", "/root/.axon_site/_ro/trn_rl_repo"):
        if _p not in sys.path:
            sys.path.insert(0, _p)

B, S, D, H, F = 2, 2048, 1024, 16, 4096
HD = D // H  # 64
EPS = 1e-5
NCORES = 8
GROUP = 4  # cores per batch group
T = (B * S) // NCORES  # 512 tokens per core
DT = D // 128  # 8 partition tiles of the model dim
FT = F // 128  # 32 partition tiles of the ff dim
JT = S // 128  # 16 key tiles per batch
VPC = H * (HD + 1)  # 1040 v' columns (64 v + 1 ones per head)
KH_ELEMS = (D // 2) * T  # kT half-block elems in one AG payload
VP_ELEMS = T * VPC  # v' block elems

# band tiles per head PAIR (max of the two heads), margin ~14 nats
PT = [1, 1, 1, 2, 4, 7, 14, 16]

QS = 8.0  # extra q scale (on top of 1/sqrt(HD))
KS = 32.0  # k scale
VS = 32.0  # v scale
ESCALE = 1.0 / (QS * KS)  # undone inside the exp activation

_CACHE = {}


def _alibi_slopes(n_heads):
    start = 2.0 ** (-(2.0 ** (-(math.log2(n_heads) - 3))))
    return np.array([start * (start**i) for i in range(n_heads)], dtype=np.float32)


def _build_nc():
    import concourse.bass as bass  # noqa: F401
    import concourse.mybir as mybir
    import concourse.tile as tile
    from concourse import bacc
    from contextlib import ExitStack

    f32 = mybir.dt.float32
    f16 = mybir.dt.float16
    bf16 = mybir.dt.bfloat16
    f8 = mybir.dt.float8e4
    Alu = mybir.AluOpType
    Act = mybir.ActivationFunctionType

    nc = bacc.Bacc("TRN2", num_devices=NCORES)

    # ---- I/O -----------------------------------------------------------
    xT = nc.declare_dram_parameter("xT", [D, T], f32, isOutput=False)
    wqT = nc.declare_dram_parameter("wqT", [D, D], bf16, isOutput=False)
    wkT = nc.declare_dram_parameter("wkT", [D, D], bf16, isOutput=False)
    wvT = nc.declare_dram_parameter("wvT", [D, D], bf16, isOutput=False)
    woT = nc.declare_dram_parameter("woT", [D, D], bf16, isOutput=False)
    w1T = nc.declare_dram_parameter("w1T", [D, F], bf16, isOutput=False)
    w2T = nc.declare_dram_parameter("w2T", [F, D], bf16, isOutput=False)
    bq = nc.declare_dram_parameter("bq", [128, DT], f32, isOutput=False)
    bk = nc.declare_dram_parameter("bk", [128, DT], f32, isOutput=False)
    bvr = nc.declare_dram_parameter("bvr", [1, D], bf16, isOutput=False)
    bo = nc.declare_dram_parameter("bo", [128, DT], f32, isOutput=False)
    b1 = nc.declare_dram_parameter("b1", [128, FT], f32, isOutput=False)
    b2 = nc.declare_dram_parameter("b2", [128, DT], f32, isOutput=False)
    alibi = nc.declare_dram_parameter("alibi", [128, JT * H], f32, isOutput=False)
    out = nc.declare_dram_parameter("out", [D, T], f32, isOutput=True)

    groups = [[0, 1, 2, 3], [4, 5, 6, 7]]

    with ExitStack() as top:
        tc = top.enter_context(tile.TileContext(nc))

        # ---- persistent pools -----------------------------------------
        const = top.enter_context(tc.tile_pool(name="const", bufs=1))
        xt_pool = top.enter_context(tc.tile_pool(name="xt_pool", bufs=DT))
        xhat_pool = top.enter_context(tc.tile_pool(name="xhat_pool", bufs=DT))
        qt_pool = top.enter_context(tc.tile_pool(name="qt_pool", bufs=DT))
        e_pool = top.enter_context(tc.tile_pool(name="e_pool", bufs=2 * sum(PT)))
        vt_pool = top.enter_context(tc.tile_pool(name="vt_pool", bufs=16))
        kt_pool = top.enter_context(tc.tile_pool(name="kt_pool", bufs=6))
        ao_pool = top.enter_context(tc.tile_pool(name="ao_pool", bufs=H // 2))
        x2_pool = top.enter_context(tc.tile_pool(name="x2_pool", bufs=DT))
        ff1_pool = top.enter_context(tc.tile_pool(name="ff1_pool", bufs=FT))
        wt_pool = top.enter_context(tc.tile_pool(name="wt_pool", bufs=12))
        wt2_pool = top.enter_context(tc.tile_pool(name="wt2_pool", bufs=12))
        dram = top.enter_context(tc.tile_pool(name="dram", bufs=1, space="DRAM"))

        # ---- warm-up collective: absorbs CC setup + inter-core skew so
        # the first real AllGather runs at wire speed ---------------------
        cc_wu_in = dram.tile([128], f32)
        cc_wu_out = dram.tile([GROUP, 128], f32)
        wu_sb = const.tile([1, 128], f32)
        nc.vector.memset(wu_sb, 0.0)
        nc.sync.dma_start(out=cc_wu_in[:].rearrange("(o t) -> o t", o=1), in_=wu_sb)
        nc.gpsimd.collective_compute(
            "AllGather",
            Alu.bypass,
            replica_groups=groups,
            ins=[cc_wu_in[:]],
            outs=[cc_wu_out[:]],
        )

        # ---- x tiles first on the sync queue (LN1 gates everything) ----
        xt_tiles = []
        for t in range(DT):
            xt = xt_pool.tile([128, T], f32, name="xt", tag="xt")
            nc.sync.dma_start(out=xt, in_=xT[t * 128:(t + 1) * 128, :])
            xt_tiles.append(xt)

        # ---- constants (DMAs on the vector queue, off the x path) ------
        ones_col = const.tile([128, 1], f32)  # stats matmul lhsT
        nc.vector.memset(ones_col, 1.0)
        ones_row = const.tile([1, 128], f32)  # row->[128,*] broadcast lhsT
        nc.vector.memset(ones_row, 1.0)
        ones_hrow = const.tile([HD + 1, 64], f16)  # denom broadcast lhsT (row 64)
        nc.vector.memset(ones_hrow, 1.0)
        ones_rowb = const.tile([1, 128], bf16)  # v-bias matmul lhsT
        nc.vector.memset(ones_rowb, 1.0)
        eps_ap = const.tile([1, 1], f32)
        nc.vector.memset(eps_ap, EPS)
        zero64 = const.tile([HD + 1, 1], f32)  # zero bias AP at base 64
        nc.vector.memset(zero64, 0.0)

        alibi_sb = const.tile([128, JT * H], f32)
        nc.scalar.dma_start(out=alibi_sb, in_=alibi[:, :])
        bq_sb = const.tile([128, DT], f32)
        nc.scalar.dma_start(out=bq_sb, in_=bq[:, :])
        bk_sb = const.tile([128, DT], f32)
        nc.scalar.dma_start(out=bk_sb, in_=bk[:, :])
        bvr_sb = const.tile([1, D], bf16)
        nc.scalar.dma_start(out=bvr_sb, in_=bvr[:, :])
        bo_sb = const.tile([128, DT], f32)
        nc.scalar.dma_start(out=bo_sb, in_=bo[:, :])
        b1_sb = const.tile([128, FT], f32)
        nc.scalar.dma_start(out=b1_sb, in_=b1[:, :])
        b2_sb = const.tile([128, DT], f32)
        nc.scalar.dma_start(out=b2_sb, in_=b2[:, :])

        # ---- AG bounce buffers ----------------------------------------
        cc_in_k0 = dram.tile([KH_ELEMS], f8)
        cc_in_k1 = dram.tile([KH_ELEMS], f8)
        cc_in_v = dram.tile([VP_ELEMS], f8)
        cc_out_k0 = dram.tile([GROUP, KH_ELEMS], f8)
        cc_out_k1 = dram.tile([GROUP, KH_ELEMS], f8)
        cc_out_v = dram.tile([GROUP, VP_ELEMS], f8)
        ccin_k = [
            cc_in_k0[:].rearrange("(d t) -> d t", t=T),
            cc_in_k1[:].rearrange("(d t) -> d t", t=T),
        ]
        ccin_v = cc_in_v[:].rearrange("(t c) -> t c", c=VPC)

        # ---------------------------------------------------------------
        def layernorm_T(xt_tiles, psum_pool, tmp_pool, out_pool, out_dtype):
            """Transposed layernorm: per-token (free axis) mean/var via
            ones-matmuls over the 8 partition tiles; returns 8 normalized
            tiles in out_dtype."""
            ps_sum = psum_pool.tile([1, T], f32, name="ps_sum", tag="ln")
            ps_sq = psum_pool.tile([1, T], f32, name="ps_sq", tag="ln")
            for t in range(DT):
                sq = tmp_pool.tile([128, T], f32, name="sq", tag="sq")
                nc.vector.tensor_mul(sq, xt_tiles[t], xt_tiles[t])
                nc.tensor.matmul(
                    ps_sum, lhsT=ones_col, rhs=xt_tiles[t],
                    start=(t == 0), stop=(t == DT - 1),
                )
                nc.tensor.matmul(
                    ps_sq, lhsT=ones_col, rhs=sq,
                    start=(t == 0), stop=(t == DT - 1),
                )
            mu = tmp_pool.tile([1, T], f32, name="mu", tag="lnrow")
            nc.scalar.mul(out=mu, in_=ps_sum, mul=1.0 / D)
            e2 = tmp_pool.tile([1, T], f32, name="e2", tag="lnrow")
            nc.scalar.mul(out=e2, in_=ps_sq, mul=1.0 / D)
            var = tmp_pool.tile([1, T], f32, name="var", tag="lnrow")
            nc.vector.tensor_mul(var, mu, mu)
            nc.vector.tensor_sub(var, e2, var)
            lnv = tmp_pool.tile([1, T], f32, name="lnv", tag="lnrow")
            nc.scalar.activation(lnv, var, Act.Ln, bias=eps_ap, scale=1.0)
            rr = tmp_pool.tile([1, T], f32, name="rr", tag="lnrow")
            nc.scalar.activation(rr, lnv, Act.Exp, scale=-0.5)
            br = tmp_pool.tile([1, T], f32, name="br", tag="lnrow")
            nc.vector.tensor_mul(br, mu, rr)
            ps_a = psum_pool.tile([128, T], f32, name="ps_a", tag="ln")
            ps_b = psum_pool.tile([128, T], f32, name="ps_b", tag="ln")
            nc.tensor.matmul(ps_a, lhsT=ones_row, rhs=rr, start=True, stop=True)
            nc.tensor.matmul(ps_b, lhsT=ones_row, rhs=br, start=True, stop=True)
            a_sb = tmp_pool.tile([128, T], f32, name="a_sb", tag="lnab")
            b_sb = tmp_pool.tile([128, T], f32, name="b_sb", tag="lnab")
            nc.vector.tensor_copy(a_sb, ps_a)
            nc.vector.tensor_copy(b_sb, ps_b)
            outs = []
            for t in range(DT):
                tmp = tmp_pool.tile([128, T], f32, name="lntmp", tag="sq")
                nc.vector.tensor_mul(tmp, xt_tiles[t], a_sb)
                xh = out_pool.tile([128, T], out_dtype, name="xh", tag="xh")
                nc.vector.tensor_sub(xh, tmp, b_sb)
                outs.append(xh)
            return outs

        # ---- phase 1: LN1 + projections -------------------------------
        with tc.tile_pool(name="ln1_ps", bufs=2, space="PSUM") as ln_ps, \
             tc.tile_pool(name="ln1_tmp", bufs=3) as ln_tmp, \
             tc.tile_pool(name="proj_ps", bufs=6, space="PSUM") as proj_ps, \
             tc.tile_pool(name="ev_tmp", bufs=3) as ev_tmp:
            xhat = layernorm_T(xt_tiles, ln_ps, ln_tmp, xhat_pool, bf16)

            # ---- k projection -> bounce, one AllGather per og half ----
            for og in range(2):
                pss = [
                    proj_ps.tile([128, T], f32, name=f"ps_k{oi}", tag="proj", bufs=6)
                    for oi in range(4)
                ]
                for kt in range(DT):
                    w = wt_pool.tile([128, 512], bf16, name="wk_t", tag="wsl")
                    nc.sync.dma_start(
                        out=w,
                        in_=wkT[kt * 128:(kt + 1) * 128, og * 512:(og + 1) * 512],
                    )
                    for oi in range(4):
                        nc.tensor.matmul(
                            pss[oi], lhsT=w[:, oi * 128:(oi + 1) * 128],
                            rhs=xhat[kt], start=(kt == 0), stop=(kt == DT - 1),
                        )
                for oi in range(4):
                    ot = og * 4 + oi
                    kt_sb = ev_tmp.tile([128, T], f8, name="kt_sb", tag="ev")
                    nc.vector.tensor_scalar(
                        out=kt_sb, in0=pss[oi], scalar1=bk_sb[:, ot:ot + 1],
                        scalar2=None, op0=Alu.add,
                    )
                    nc.scalar.dma_start(
                        out=ccin_k[og][oi * 128:(oi + 1) * 128, :], in_=kt_sb
                    )
                nc.gpsimd.collective_compute(
                    "AllGather",
                    Alu.bypass,
                    replica_groups=groups,
                    ins=[(cc_in_k0 if og == 0 else cc_in_k1)[:]],
                    outs=[(cc_out_k0 if og == 0 else cc_out_k1)[:]],
                )

            # ---- v projection (token-partition layout) -> bounce ------
            for oh in range(2):
                psv = [
                    proj_ps.tile([128, 512], f32, name=f"ps_v{it}", tag="proj", bufs=6)
                    for it in range(4)
                ]
                for kt in range(DT):
                    w = wt_pool.tile([128, 512], bf16, name="wv_t", tag="wsl")
                    nc.sync.dma_start(
                        out=w,
                        in_=wvT[kt * 128:(kt + 1) * 128, oh * 512:(oh + 1) * 512],
                    )
                    for it in range(4):
                        nc.tensor.matmul(
                            psv[it], lhsT=xhat[kt][:, it * 128:(it + 1) * 128],
                            rhs=w, start=(kt == 0), stop=False,
                        )
                for it in range(4):
                    nc.tensor.matmul(
                        psv[it], lhsT=ones_rowb,
                        rhs=bvr_sb[:, oh * 512:(oh + 1) * 512],
                        start=False, stop=True,
                    )
                    # stage with per-head ones columns interleaved: [t, 8, 65]
                    vst = ev_tmp.tile([128, 8 * (HD + 1)], f8, name="vst", tag="vst")
                    vst3 = vst.rearrange("t (h c) -> t h c", c=HD + 1)
                    nc.vector.tensor_copy(
                        vst3[:, :, 0:HD],
                        psv[it].rearrange("t (h c) -> t h c", c=HD),
                    )
                    nc.vector.memset(vst3[:, :, HD:HD + 1], 1.0)
                    nc.scalar.dma_start(
                        out=ccin_v[it * 128:(it + 1) * 128,
                                   oh * 8 * (HD + 1):(oh + 1) * 8 * (HD + 1)],
                        in_=vst,
                    )
            nc.gpsimd.collective_compute(
                "AllGather",
                Alu.bypass,
                replica_groups=groups,
                ins=[cc_in_v[:]],
                outs=[cc_out_v[:]],
            )

            # ---- q projection (local only) ----------------------------
            qt_tiles = []
            for og in range(2):
                psq = [
                    proj_ps.tile([128, T], f32, name=f"ps_q{oi}", tag="proj",
                                 bufs=6)
                    for oi in range(4)
                ]
                for kt in range(DT):
                    w = wt_pool.tile([128, 512], bf16, name="wq_t", tag="wsl")
                    nc.sync.dma_start(
                        out=w,
                        in_=wqT[kt * 128:(kt + 1) * 128, og * 512:(og + 1) * 512],
                    )
                    for oi in range(4):
                        nc.tensor.matmul(
                            psq[oi], lhsT=w[:, oi * 128:(oi + 1) * 128],
                            rhs=xhat[kt], start=(kt == 0), stop=(kt == DT - 1),
                        )
                for oi in range(4):
                    ot = og * 4 + oi
                    qt = qt_pool.tile([128, T], f8, name="qt", tag="qt")
                    nc.vector.tensor_scalar(
                        out=qt, in0=psq[oi], scalar1=bq_sb[:, ot:ot + 1],
                        scalar2=None, op0=Alu.add,
                    )
                    qt_tiles.append(qt)

        # ---- phase 2a: scores + exp for every pair (e stashed fp8) ----
        e_tiles = {}  # (hp, jt, parity) -> fp8 [128, T]
        with tc.tile_pool(name="sc_ps", bufs=4, space="PSUM") as sc_ps:
            for hp in range(H // 2):
                nt = PT[hp]
                jt0 = JT - nt
                cko = cc_out_k0 if hp < 4 else cc_out_k1
                row0 = (hp % 4) * 128
                ks = {}
                for r in range(jt0 // 4, GROUP):
                    ck = cko[r, :].rearrange("(d t) -> d t", t=T)
                    ksl = kt_pool.tile([128, T], f8, name="ks", tag="ktt")
                    nc.sync.dma_start(out=ksl, in_=ck[row0:row0 + 128, :])
                    ks[r] = ksl
                for jt in range(jt0, JT):
                    r, jl = jt // 4, jt % 4
                    kt_t = ks[r][:, jl * 128:(jl + 1) * 128]
                    ps_se = sc_ps.tile([128, T], f32, name="ps_se", tag="sc")
                    nc.tensor.matmul(
                        ps_se, lhsT=kt_t[0:64, :], rhs=qt_tiles[hp][0:64, :],
                        start=True, stop=True,
                    )
                    ps_so = sc_ps.tile([128, T], f32, name="ps_so", tag="sc")
                    nc.tensor.matmul(
                        ps_so, lhsT=kt_t[64:128, :], rhs=qt_tiles[hp][64:128, :],
                        start=True, stop=True,
                    )
                    e_e = e_pool.tile([128, T], f8, name="e_e", tag="e")
                    nc.scalar.activation(
                        e_e, ps_se, Act.Exp,
                        bias=alibi_sb[:, jt * H + 2 * hp:jt * H + 2 * hp + 1],
                        scale=ESCALE,
                    )
                    e_o = e_pool.tile([128, T], f8, name="e_o", tag="e")
                    nc.scalar.activation(
                        e_o, ps_so, Act.Exp,
                        bias=alibi_sb[:, jt * H + 2 * hp + 1:jt * H + 2 * hp + 2],
                        scale=ESCALE,
                    )
                    e_tiles[(hp, jt, 0)] = e_e
                    e_tiles[(hp, jt, 1)] = e_o

        # ---- phase 2b: bulk-load gathered v' --------------------------
        v_tiles = []  # global key tile jt -> [128, VPC] fp8
        for r in range(GROUP):
            cv = cc_out_v[r, :].rearrange("(t c) -> t c", c=VPC)
            for jl in range(4):
                vt = vt_pool.tile([128, VPC], f8, name="vt", tag="vt")
                nc.sync.dma_start(out=vt, in_=cv[jl * 128:(jl + 1) * 128, :])
                v_tiles.append(vt)

        # ---- phase 2c: AV + normalize ---------------------------------
        ao_tiles = []  # 8 tiles [128, T] bf16, head-pair-major
        with tc.tile_pool(name="av_ps", bufs=6, space="PSUM") as av_ps, \
             tc.tile_pool(name="rb_ps", bufs=2, space="PSUM") as rb_ps, \
             tc.tile_pool(name="dn_sb", bufs=4) as dn_sb, \
             tc.tile_pool(name="rb_sb", bufs=4) as rb_sbp:
            for hp in range(H // 2):
                nt = PT[hp]
                jt0 = JT - nt
                ps_e = av_ps.tile([HD + 1, T], f32, name="ps_e", tag="av")
                ps_o = av_ps.tile([HD + 1, T], f32, name="ps_o", tag="av")
                for jt in range(jt0, JT):
                    vt = v_tiles[jt]
                    nc.tensor.matmul(
                        ps_e,
                        lhsT=vt[:, (2 * hp) * (HD + 1):(2 * hp + 1) * (HD + 1)],
                        rhs=e_tiles[(hp, jt, 0)],
                        start=(jt == jt0), stop=(jt == JT - 1),
                    )
                    nc.tensor.matmul(
                        ps_o,
                        lhsT=vt[:, (2 * hp + 1) * (HD + 1):(2 * hp + 2) * (HD + 1)],
                        rhs=e_tiles[(hp, jt, 1)],
                        start=(jt == jt0), stop=(jt == JT - 1),
                    )
                ao_pair = ao_pool.tile([128, T], bf16, name="ao_pair", tag="ao")
                for which, ps in ((0, ps_e), (1, ps_o)):
                    # reciprocal as exp(-ln(x)) on ACT (idle here; DVE
                    # InstReciprocal measures ~3.3us, this chain ~1.2us)
                    lnd = dn_sb.tile([HD + 1, T], f32, name="lnd", tag="lnd")
                    nc.scalar.activation(
                        lnd[HD:HD + 1, :], ps[HD:HD + 1, :], Act.Ln,
                        bias=zero64[HD:HD + 1, :], scale=1.0,
                    )
                    rdenh = dn_sb.tile([HD + 1, T], f16, name="rdenh", tag="rdenh")
                    nc.scalar.activation(
                        rdenh[HD:HD + 1, :], lnd[HD:HD + 1, :], Act.Exp,
                        scale=-1.0,
                    )
                    ps_rb = rb_ps.tile([64, T], f32, name="ps_rb", tag="rb")
                    nc.tensor.matmul(
                        ps_rb, lhsT=ones_hrow[HD:HD + 1, :],
                        rhs=rdenh[HD:HD + 1, :],
                        start=True, stop=True,
                    )
                    rb = rb_sbp.tile([64, T], f32, name="rb", tag="rbs")
                    nc.vector.tensor_copy(rb, ps_rb)
                    if which == 0:
                        nc.vector.tensor_mul(ao_pair[0:HD, :], ps[0:HD, :], rb)
                    else:
                        ao_tmp = rb_sbp.tile(
                            [64, T], bf16, name="ao_tmp", tag="aot"
                        )
                        nc.vector.tensor_mul(ao_tmp, ps[0:HD, :], rb)
                        nc.sync.dma_start(out=ao_pair[HD:128, :], in_=ao_tmp)
                ao_tiles.append(ao_pair)

        # ---- phase 3: output projection + residual --------------------
        x2_tiles = [None] * DT
        with tc.tile_pool(name="wo_ps", bufs=4, space="PSUM") as wo_ps:
            for og in range(2):
                psw = [
                    wo_ps.tile([128, T], f32, name=f"ps_wo{oi}", tag="wo", bufs=8)
                    for oi in range(4)
                ]
                for hp in range(H // 2):
                    w = wt2_pool.tile([128, 512], bf16, name="wo_t", tag="wsl")
                    nc.scalar.dma_start(
                        out=w,
                        in_=woT[hp * 128:(hp + 1) * 128, og * 512:(og + 1) * 512],
                    )
                    for oi in range(4):
                        nc.tensor.matmul(
                            psw[oi], lhsT=w[:, oi * 128:(oi + 1) * 128],
                            rhs=ao_tiles[hp], start=(hp == 0),
                            stop=(hp == H // 2 - 1),
                        )
                for oi in range(4):
                    ot = og * 4 + oi
                    x2 = x2_pool.tile([128, T], f32, name="x2", tag="x2")
                    nc.vector.tensor_scalar(
                        out=x2, in0=psw[oi], scalar1=bo_sb[:, ot:ot + 1],
                        scalar2=None, op0=Alu.add,
                    )
                    nc.vector.tensor_add(x2, x2, xt_tiles[ot])
                    x2_tiles[ot] = x2

        # ---- phase 4: LN2 + FFN ---------------------------------------
        with tc.tile_pool(name="ln2_ps", bufs=2, space="PSUM") as ln2_ps, \
             tc.tile_pool(name="ln2_tmp", bufs=3) as ln2_tmp, \
             tc.tile_pool(name="xh2_pool", bufs=DT) as xh2_pool:
            xhat2 = layernorm_T(x2_tiles, ln2_ps, ln2_tmp, xh2_pool, bf16)

            ff1_tiles = []
            with tc.tile_pool(name="f1_ps", bufs=3, space="PSUM") as f1_ps:
                for fg in range(FT // 4):
                    psf = [
                        f1_ps.tile([128, T], f32, name=f"ps_f1{fi}", tag="f1",
                                   bufs=6)
                        for fi in range(4)
                    ]
                    for kt in range(DT):
                        w = wt2_pool.tile([128, 512], bf16, name="w1_t", tag="wsl")
                        nc.scalar.dma_start(
                            out=w,
                            in_=w1T[kt * 128:(kt + 1) * 128,
                                    fg * 512:(fg + 1) * 512],
                        )
                        for fi in range(4):
                            nc.tensor.matmul(
                                psf[fi], lhsT=w[:, fi * 128:(fi + 1) * 128],
                                rhs=xhat2[kt],
                                start=(kt == 0), stop=(kt == DT - 1),
                            )
                    for fi in range(4):
                        ft = fg * 4 + fi
                        f1 = ff1_pool.tile([128, T], bf16, name="f1", tag="f1s")
                        # relu(x + b1) fused on DVE (keeps ACT free)
                        nc.vector.tensor_scalar(
                            out=f1, in0=psf[fi], scalar1=b1_sb[:, ft:ft + 1],
                            scalar2=0.0, op0=Alu.add, op1=Alu.max,
                        )
                        ff1_tiles.append(f1)

            # second FFN matmul: two groups of 4 output tiles, accumulate
            # over all 32 f-tiles with w2 streamed once per group
            with tc.tile_pool(name="f2_ps", bufs=4, space="PSUM") as f2_ps:
                for og in range(2):
                    pss = []
                    for oi in range(4):
                        ps = f2_ps.tile([128, T], f32, name="ps_f2", tag="f2")
                        pss.append(ps)
                    for ft in range(FT):
                        w = wt2_pool.tile([128, 512], bf16, name="w2_t", tag="wsl")
                        nc.scalar.dma_start(
                            out=w,
                            in_=w2T[ft * 128:(ft + 1) * 128,
                                    og * 512:(og + 1) * 512],
                        )
                        for oi in range(4):
                            nc.tensor.matmul(
                                pss[oi], lhsT=w[:, oi * 128:(oi + 1) * 128],
                                rhs=ff1_tiles[ft],
                                start=(ft == 0), stop=(ft == FT - 1),
                            )
                    for oi in range(4):
                        ot = og * 4 + oi
                        y = ln2_tmp.tile([128, T], f32, name="y", tag="sq")
                        nc.vector.tensor_scalar(
                            out=y, in0=pss[oi], scalar1=b2_sb[:, ot:ot + 1],
                            scalar2=None, op0=Alu.add,
                        )
                        nc.vector.tensor_add(y, y, x2_tiles[ot])
                        nc.sync.dma_start(
                            out=out[ot * 128:(ot + 1) * 128, :], in_=y
                        )

    nc.compile()
    return nc


def _get_nc():
    if "nc" not in _CACHE:
        _CACHE["nc"] = _build_nc()
    return _CACHE["nc"]


def kernel(x, Wq, Wk, Wv, Wo, bo, W1, b1, W2, b2, g1, be1, g2, be2):
    import ml_dtypes

    f32 = np.float32
    bf = ml_dtypes.bfloat16
    x = np.asarray(x, f32)
    Wq = np.asarray(Wq, f32); Wk = np.asarray(Wk, f32)
    Wv = np.asarray(Wv, f32); Wo = np.asarray(Wo, f32)
    W1 = np.asarray(W1, f32); W2 = np.asarray(W2, f32)
    bo = np.asarray(bo, f32); b1 = np.asarray(b1, f32); b2 = np.asarray(b2, f32)
    g1 = np.asarray(g1, f32); be1 = np.asarray(be1, f32)
    g2 = np.asarray(g2, f32); be2 = np.asarray(be2, f32)

    scale = 1.0 / math.sqrt(HD)
    wqT = np.ascontiguousarray((Wq * g1[None, :] * (scale * QS)).T).astype(bf)
    wkT = np.ascontiguousarray((Wk * g1[None, :] * KS).T).astype(bf)
    wvT = np.ascontiguousarray((Wv * g1[None, :] * VS).T).astype(bf)
    woT = np.ascontiguousarray(Wo.T / VS).astype(bf)
    w1T = np.ascontiguousarray((W1 * g2[None, :]).T).astype(bf)
    w2T = np.ascontiguousarray(W2.T).astype(bf)
    bq_v = (be1 @ Wq.T) * scale * QS
    bk_v = (be1 @ Wk.T) * KS
    bv_v = (be1 @ Wv.T) * VS
    b1_v = b1 + be2 @ W1.T

    def cols(v, nt):  # (nt*128,) -> (128, nt) [partition, tile]
        return np.ascontiguousarray(v.reshape(nt, 128).T).astype(f32)

    slopes = _alibi_slopes(H)
    j = np.arange(S, dtype=f32)
    vals = slopes[:, None] * (j[None, :] - (S - 1))  # (H, S)
    alibi = np.ascontiguousarray(
        vals.reshape(H, JT, 128).transpose(2, 1, 0).reshape(128, JT * H)
    ).astype(f32)

    xt_flat = x.reshape(B * S, D)
    base = {
        "wqT": wqT, "wkT": wkT, "wvT": wvT, "woT": woT,
        "w1T": w1T, "w2T": w2T,
        "bq": cols(bq_v, DT), "bk": cols(bk_v, DT),
        "bvr": np.ascontiguousarray(bv_v[None, :]).astype(bf),
        "bo": cols(bo, DT), "b1": cols(b1_v, FT), "b2": cols(b2, DT),
        "alibi": alibi,
    }
    in_maps = []
    for c in range(NCORES):
        m = dict(base)
        m["xT"] = np.ascontiguousarray(xt_flat[c * T:(c + 1) * T].T).astype(f32)
        in_maps.append(m)

    from concourse.bass_utils import run_bass_kernel_spmd

    nc = _get_nc()
    res = run_bass_kernel_spmd(nc, in_maps, core_ids=list(range(NCORES)))
    _CACHE["last_result"] = res
    outs = [r["out"] for r in res.results]  # each (D, T)
    full = np.empty((B * S, D), dtype=f32)
    for c in range(NCORES):
        full[c * T:(c + 1) * T] = outs[c].T
    return full.reshape(B, S, D)
